# revision 1
# baseline (speedup 1.0000x reference)
"""Trainium2 Bass kernel for the GroupNorm->QKV->MHA->proj residual attention block.

Problem shapes (hardcoded): x [4, 128, 64, 64] f32, HEADS=4, GROUPS=32, L=4096.

Sharding: 16 (batch, head) pairs over 8 cores -> each core handles one batch and
two heads.  Each core computes GN + its heads' qkv + attention + a partial
projection over its 64 attention channels (+ 0.5*(x + b_proj)); the host sums
the two partials of each batch.

All matmuls use the full 128x128 PE tile (small contractions are zero-padded to
K=128 - matmul time scales with N only, and a uniform tile mode avoids the
PE reconfiguration drains that also keep the HAM clock-gate stuck at 1.2 GHz).
"""

import functools
import sys

sys.path.insert(0, "/opt/trn_rl_repo")

import numpy as np
import ml_dtypes

import concourse.bass as bass
import concourse.bacc as bacc
import concourse.tile as tile
from concourse import mybir
from concourse.bass_utils import run_bass_kernel_spmd

F32 = mybir.dt.float32
BF16 = mybir.dt.bfloat16

B, C, H, W = 4, 128, 64, 64
HEADS = 4
GROUPS = 32
EPS = 1e-5
L = H * W          # 4096
CH = C // HEADS    # 32
NCORES = 8
NCHUNK = L // 512  # 8 column chunks of 512
NST = L // 128     # 32 s-tiles of 128

# s-tile group sizes per chunk: exp widths 1536 + one 1024 tail (PSUM: 2x3 + 2x1 banks)
UNITS = [3, 3, 3, 3, 3, 3, 3, 3, 3, 3, 2]
assert sum(UNITS) == NST

USE_DIVIDE = False  # tensor_tensor divide fails the DVE ISA check; use reciprocal + mult


def _bcast_ap(src, parts):
    """Partition-broadcast access pattern: read a [1, N] slice `parts` times."""
    return bass.AP(
        tensor=src.tensor,
        offset=src.offset,
        ap=[[0, parts]] + [list(d) for d in src.ap[1:]],
    )


def _body(tc, x, wqk, wv, bqk, bv, wp, hb, gmat, rs_d, part):
    nc = tc.nc
    AF = mybir.ActivationFunctionType
    ALU = mybir.AluOpType

    from contextlib import ExitStack

    with ExitStack() as ctx:
        const = ctx.enter_context(tc.tile_pool(name="const", bufs=1))
        big = ctx.enter_context(tc.tile_pool(name="big", bufs=1))
        pbuf = ctx.enter_context(tc.tile_pool(name="pbuf", bufs=4))
        small = ctx.enter_context(tc.tile_pool(name="small", bufs=4))
        spsum = ctx.enter_context(tc.tile_pool(name="spsum", bufs=2, space="PSUM"))
        aux = ctx.enter_context(tc.tile_pool(name="aux", bufs=2, space="PSUM"))

        _spn = [0]

        def sp_tile(k):  # rotating wide psum slots for matmul outputs
            _spn[0] += 1
            return spsum.tile([C, 1536], F32, tag="sp", name=f"sp_{_spn[0]}")

        # persistent big tiles; memsets first so they hide under the x load
        x_sb = big.tile([C, L], F32, tag="x")
        x_bf = big.tile([C, L], BF16, tag="xbf")
        vt_all = big.tile([C, NST, C], BF16, tag="vt")
        araw = big.tile([C, 16 * 512], F32, tag="araw")
        a_acc = big.tile([C, L], BF16, tag="aacc")
        nc.vector.memset(a_acc, 0.0)
        nc.vector.memset(vt_all[:, :, 32:33], 1.0)
        nc.vector.memset(vt_all[:, :, 96:97], 1.0)
        nc.vector.memset(vt_all[:, :, 33:64], 0.0)
        nc.vector.memset(vt_all[:, :, 97:128], 0.0)

        stats = small.tile([C, NCHUNK, 6], F32, tag="stats")
        dma_engines = [nc.sync, nc.scalar, nc.gpsimd]
        for c in range(NCHUNK):
            dma_engines[c % 3].dma_start(
                out=x_sb[:, 512 * c : 512 * (c + 1)], in_=x[:, 512 * c : 512 * (c + 1)]
            )
            nc.vector.bn_stats(out=stats[:, c, :], in_=x_sb[:, 512 * c : 512 * (c + 1)])
            nc.vector.tensor_copy(
                out=x_bf[:, 512 * c : 512 * (c + 1)],
                in_=x_sb[:, 512 * c : 512 * (c + 1)],
            )

        # ---- constants into SBUF ----
        wqk_sb = const.tile([C, 512], BF16, tag="wqk")
        nc.sync.dma_start(out=wqk_sb, in_=wqk)
        wv_sb = const.tile([C, 96], BF16, tag="wv")
        nc.sync.dma_start(out=wv_sb, in_=wv)
        bqk_sb = const.tile([C, 4], F32, tag="bqk")
        nc.sync.dma_start(out=bqk_sb, in_=bqk)
        wps_sb = const.tile([C, C], BF16, tag="wps")
        nc.sync.dma_start(out=wps_sb, in_=wp)
        hb_sb = const.tile([C, 1], F32, tag="hb")
        nc.sync.dma_start(out=hb_sb, in_=hb)
        gmat_sb = const.tile([C, C], F32, tag="gmat")
        nc.sync.dma_start(out=gmat_sb, in_=gmat)

        # ---- GroupNorm statistics -> per-channel mean and rstd ----
        mv = small.tile([C, 2], F32, tag="mv")
        nc.vector.bn_aggr(out=mv, in_=stats)
        ms = small.tile([C, 2], F32, tag="ms")  # [mean, var + mean^2]
        nc.vector.tensor_copy(out=ms[:, 0:1], in_=mv[:, 0:1])
        nc.vector.tensor_scalar(
            out=ms[:, 1:2],
            in0=mv[:, 0:1],
            scalar1=mv[:, 0:1],
            scalar2=mv[:, 1:2],
            op0=ALU.mult,
            op1=ALU.add,
        )
        # group-average + broadcast via 0.25-blocked matmul
        gps = aux.tile([C, 2], F32, tag="ap")
        nc.tensor.matmul(gps, lhsT=gmat_sb, rhs=ms, start=True, stop=True)
        gsb = small.tile([C, 2], F32, tag="gsb")  # [gmean, gE2]
        nc.vector.tensor_copy(out=gsb, in_=gps)
        gv = small.tile([C, 1], F32, tag="gv")  # gmean^2 - gE2 = -gvar
        nc.vector.tensor_scalar(
            out=gv,
            in0=gsb[:, 0:1],
            scalar1=gsb[:, 0:1],
            scalar2=gsb[:, 1:2],
            op0=ALU.mult,
            op1=ALU.subtract,
        )
        rstd = small.tile([C, 1], F32, tag="rstd")
        epst = small.tile([C, 1], F32, tag="epst")
        nc.vector.memset(epst, EPS)
        nc.scalar.activation(out=rstd, in_=gv, func=AF.Ln, bias=epst, scale=-1.0)
        nc.scalar.activation(out=rstd, in_=rstd, func=AF.Exp, scale=-0.5)
        gmb = small.tile([C, 1], BF16, tag="gmb")
        nc.vector.tensor_copy(out=gmb, in_=gsb[:, 0:1])

        # ---- fold the normalization into the projection weights ----
        # q = W (rstd*(x-mean)) + b = (W*rstd) x + (b - (W*rstd) mean)
        wqk2 = const.tile([C, 512], BF16, tag="wqk2")
        nc.vector.tensor_scalar_mul(out=wqk2, in0=wqk_sb, scalar1=rstd)
        wv2 = const.tile([C, 96], BF16, tag="wv2")
        nc.vector.tensor_scalar_mul(out=wv2, in0=wv_sb, scalar1=rstd)
        bqk2 = const.tile([C, 4], F32, tag="bqk2")
        for blk in range(4):
            pc = aux.tile([C, 1], F32, tag="ap", name=f"pc_{blk}")
            nc.tensor.matmul(
                pc, lhsT=wqk2[:, 128 * blk : 128 * (blk + 1)], rhs=gmb, start=True, stop=True
            )
            nc.vector.tensor_sub(bqk2[:, blk : blk + 1], bqk_sb[:, blk : blk + 1], pc)
        # v mean-correction, folded through softmax into the projection bias
        pcv = aux.tile([C, 1], F32, tag="ap", name="pcv")
        nc.tensor.matmul(pcv[0:96, :], lhsT=wv2, rhs=gmb, start=True, stop=True)
        cv_sb = small.tile([C, 1], BF16, tag="cv")
        nc.vector.memset(cv_sb, 0.0)
        nc.vector.tensor_copy(out=cv_sb[0:96, :], in_=pcv[0:96, :])
        pcp = aux.tile([C, 1], F32, tag="ap", name="pcp")
        nc.tensor.matmul(pcp, lhsT=wps_sb, rhs=cv_sb, start=True, stop=True)
        hb2 = small.tile([C, 1], F32, tag="hb2")
        nc.vector.tensor_sub(hb2, hb_sb, pcp)

        # ---- q/k projections ----
        # rows 0-31 = [q | k], rows 32-127 = 0 (K padded to 128)
        qk = [
            big.tile([C, 2 * L], BF16, tag="qk0", name="qk0"),
            big.tile([C, 2 * L], BF16, tag="qk1", name="qk1"),
        ]

        def qk_mm_one(h, t, cc):
            pq = sp_tile(0)
            nc.tensor.matmul(
                pq[:, 0:512],
                lhsT=wqk2[:, 128 * (2 * h + t) : 128 * (2 * h + t + 1)],
                rhs=x_bf[:, 512 * cc : 512 * (cc + 1)],
                start=True,
                stop=True,
            )
            nc.vector.tensor_scalar_add(
                out=qk[h][:, L * t + 512 * cc : L * t + 512 * (cc + 1)],
                in0=pq[:, 0:512],
                scalar1=bqk2[:, 2 * h + t : 2 * h + t + 1],
            )

        # h0 needs all of k and q-chunk 0 before its attention starts; the other
        # q chunks are emitted just-in-time, and all of h1's q/k as background
        # work spread through h0's attention stream.
        for cc in range(NCHUNK):
            qk_mm_one(0, 1, cc)
        qk_mm_one(0, 0, 0)
        qk_mm_one(0, 0, 1)

        # ---- v^T tiles (both heads) with ones columns for the softmax rowsum ----
        # cols per l-tile: [v_h0 (0:32) | 1 (32) | junk | v_h1 (64:96) | 1 (96) | junk]
        # (junk columns produce PSUM rows the A-evac never reads; the v bias and
        #  mean-correction fold through softmax into the projection bias)
        # The matmuls are streamed into the first attention chunk via front_work.

        def vt_group(g):  # 8 l-tiles per psum slot
            pv = sp_tile(g)
            for e in range(8):
                i = 8 * g + e
                nc.tensor.matmul(
                    pv[:, 128 * e : 128 * e + 96],
                    lhsT=x_bf[:, 128 * i : 128 * (i + 1)],
                    rhs=wv2,
                    start=True,
                    stop=True,
                )
            pv3 = pv[:, 0:1024].rearrange("p (g n) -> p g n", n=128)
            nc.vector.tensor_copy(out=vt_all[:, 8 * g : 8 * (g + 1), 0:CH], in_=pv3[:, :, 0:CH])
            nc.vector.tensor_copy(
                out=vt_all[:, 8 * g : 8 * (g + 1), 64:96], in_=pv3[:, :, 64:96]
            )

        from collections import deque

        front_work = deque(range(4))  # vt groups, popped inside the first chunk
        bg_work = deque()
        for cc in range(NCHUNK):
            bg_work.append((1, 1, cc))  # h1 k
        for cc in range(NCHUNK):
            bg_work.append((1, 0, cc))  # h1 q

        # ---- attention + per-chunk projection ----

        def emit_proj(j):
            # out_partial = wps.T @ a_acc + 0.5 * (x + b_proj)
            pp = aux.tile([C, 512], F32, tag="ap", name=f"pp_{j}")
            nc.tensor.matmul(
                pp[:, 0:512],
                lhsT=wps_sb,
                rhs=a_acc[:, 512 * j : 512 * (j + 1)],
                start=True,
                stop=True,
            )
            res = small.tile([C, 512], F32, tag="res")
            nc.vector.tensor_scalar(
                out=res,
                in0=x_sb[:, 512 * j : 512 * (j + 1)],
                scalar1=0.5,
                scalar2=hb2[:, 0:1],
                op0=ALU.mult,
                op1=ALU.add,
            )
            outt = small.tile([C, 512], F32, tag="outt")
            nc.vector.tensor_add(outt, pp[:, 0:512], res)
            nc.sync.dma_start(out=part[:, 512 * j : 512 * (j + 1)], in_=outt)

        for h in range(2):
            r0 = 64 * h          # valid row range for this head in A psum

            def close_chunk(aps, j):
                # evacuate A_raw fast (frees the accumulator bank) and ship the
                # rowsum row to DRAM; the normalization itself runs one chunk
                # later so these copies never queue behind a reciprocal
                k = 8 * h + j
                nc.vector.tensor_copy(
                    out=araw[r0 : r0 + 33, 512 * k : 512 * (k + 1)],
                    in_=aps[r0 : r0 + 33, :],
                )
                nc.sync.dma_start(
                    out=rs_d[k : k + 1, :],
                    in_=araw[r0 + 32 : r0 + 33, 512 * k : 512 * (k + 1)],
                )
                if j >= 1:
                    norm_chunk(j - 1)

            def norm_chunk(j):
                k = 8 * h + j
                last = h == 1 and j == NCHUNK - 1
                rsb = small.tile([C, 512], F32, tag="rsb", name=f"rsb_{h}_{j}")
                nc.sync.dma_start(
                    out=rsb[r0 : r0 + 32, :], in_=_bcast_ap(rs_d[k : k + 1, :], 32)
                )
                if last:
                    nc.scalar.activation(
                        out=rsb[r0 : r0 + 32, :], in_=rsb[r0 : r0 + 32, :], func=AF.Ln
                    )
                    nc.scalar.activation(
                        out=rsb[r0 : r0 + 32, :], in_=rsb[r0 : r0 + 32, :],
                        func=AF.Exp, scale=-1.0,
                    )
                else:
                    nc.vector.reciprocal(out=rsb[r0 : r0 + 32, :], in_=rsb[r0 : r0 + 32, :])
                nc.vector.tensor_mul(
                    a_acc[r0 : r0 + 32, 512 * j : 512 * (j + 1)],
                    araw[r0 : r0 + 32, 512 * k : 512 * (k + 1)],
                    rsb[r0 : r0 + 32, :],
                )
                if h == 1:
                    emit_proj(j)

            def flush(p):
                aps, pj, pi, pw, ppt = p
                for r in range(pw):
                    nc.tensor.matmul(
                        aps,
                        lhsT=vt_all[:, pi + r, :],
                        rhs=ppt[:, 512 * r : 512 * (r + 1)],
                        start=(pi + r == 0),
                        stop=(pi + r == NST - 1),
                    )
                if pi + pw == NST:
                    close_chunk(aps, pj)

            # software pipeline: a unit's A-matmuls are issued two units after
            # its S-matmuls + exp, so the PE never head-of-line blocks on ACT
            pending = deque()
            for j in range(NCHUNK):
                if h == 0 and j + 2 < NCHUNK:
                    qk_mm_one(0, 0, j + 2)  # q chunk, two chunks ahead
                aps = aux.tile([C, 512], F32, tag="ap", name=f"aps_{h}_{j}")
                i = 0
                for u, width in enumerate(UNITS):
                    ps = sp_tile(u)
                    for r in range(width):
                        nc.tensor.matmul(
                            ps[:, 512 * r : 512 * (r + 1)],
                            lhsT=qk[h][:, L + 128 * (i + r) : L + 128 * (i + r + 1)],
                            rhs=qk[h][:, 512 * j : 512 * (j + 1)],
                            start=True,
                            stop=True,
                        )
                    if width == 3:
                        pt = pbuf.tile([C, 1536], BF16, tag="p", name=f"p_{h}_{j}_{u}")
                    else:
                        pt = pbuf.tile([C, 512 * width], BF16, tag="pt2", name=f"pt2_{h}_{j}_{u}")
                    nc.scalar.activation(out=pt, in_=ps[:, 0 : 512 * width], func=AF.Exp)
                    if front_work:
                        vt_group(front_work.popleft())
                    pending.append((aps, j, i, width, pt))
                    if len(pending) > 2:
                        flush(pending.popleft())
                        if h == 0 and j >= 5 and bg_work:
                            qk_mm_one(*bg_work.popleft())  # h1 q/k, spread out
                    i += width
            while pending:
                flush(pending.popleft())
            while bg_work:
                qk_mm_one(*bg_work.popleft())
            norm_chunk(NCHUNK - 1)

@functools.lru_cache(maxsize=1)
def _build_program():
    nc = bacc.Bacc("TRN2", target_bir_lowering=False, debug=False, num_devices=NCORES)
    x = nc.dram_tensor("x", [C, L], F32, kind="ExternalInput").ap()
    wqk = nc.dram_tensor("wqk", [C, 512], BF16, kind="ExternalInput").ap()
    wv = nc.dram_tensor("wv", [C, 96], BF16, kind="ExternalInput").ap()
    bqk = nc.dram_tensor("bqk", [C, 4], F32, kind="ExternalInput").ap()
    bv = nc.dram_tensor("bv", [1, 2 * CH], F32, kind="ExternalInput").ap()
    wp = nc.dram_tensor("wp", [C, C], BF16, kind="ExternalInput").ap()
    hb = nc.dram_tensor("hb", [C, 1], F32, kind="ExternalInput").ap()
    gmat = nc.dram_tensor("gmat", [C, C], F32, kind="ExternalInput").ap()
    rs_d = nc.dram_tensor("rs_d", [16, 512], F32).ap()
    part = nc.dram_tensor("part", [C, L], F32, kind="ExternalOutput").ap()
    with tile.TileContext(nc) as tc:
        _body(tc, x, wqk, wv, bqk, bv, wp, hb, gmat, rs_d, part)
    nc.compile()
    return nc


def make_in_maps(inputs):
    x = np.ascontiguousarray(np.asarray(inputs["x"], np.float32))
    gamma = np.asarray(inputs["gn_gamma"], np.float32)
    beta = np.asarray(inputs["gn_beta"], np.float32)
    w_qkv = np.asarray(inputs["w_qkv"], np.float32)
    b_qkv = np.asarray(inputs["b_qkv"], np.float32)
    w_proj = np.asarray(inputs["w_proj"], np.float32)
    b_proj = np.asarray(inputs["b_proj"], np.float32)

    scale = (1.0 / np.sqrt(np.sqrt(CH))).astype(np.float32)
    Wg = w_qkv * gamma[None, :]                  # fold GN gamma
    bf = b_qkv + w_qkv @ beta                    # fold GN beta
    gmat_np = np.zeros((C, C), np.float32)
    for g in range(GROUPS):
        gmat_np[g * 4 : (g + 1) * 4, g * 4 : (g + 1) * 4] = 0.25

    in_maps = []
    for core in range(NCORES):
        b = core // 2
        pi = core % 2
        hg = [2 * pi, 2 * pi + 1]  # global head ids of local heads 0, 1

        # wqk: 4 blocks of [128 (c), 128 (M)]: [h0 q, h0 k, h1 q, h1 k];
        # each block has W.T in cols 0:32, zeros elsewhere (K padded to 128)
        wqk_np = np.zeros((C, 512), np.float32)
        bqk_np = np.zeros((C, 4), np.float32)
        for lh, g in enumerate(hg):
            qW = Wg[CH * g : CH * (g + 1)] * scale          # [32, 128]
            kW = Wg[C + CH * g : C + CH * (g + 1)] * scale
            wqk_np[:, 256 * lh : 256 * lh + 32] = qW.T
            wqk_np[:, 256 * lh + 128 : 256 * lh + 160] = kW.T
            bqk_np[0:32, 2 * lh] = bf[CH * g : CH * (g + 1)] * scale
            bqk_np[0:32, 2 * lh + 1] = bf[C + CH * g : C + CH * (g + 1)] * scale

        wv_np = np.zeros((C, 96), np.float32)
        bv_np = np.zeros((1, 2 * CH), np.float32)
        for lh, g in enumerate(hg):
            wv_np[:, 64 * lh : 64 * lh + CH] = Wg[2 * C + CH * g : 2 * C + CH * (g + 1)].T
            bv_np[0, CH * lh : CH * (lh + 1)] = bf[2 * C + CH * g : 2 * C + CH * (g + 1)]

        # wps rows 0:32 = w_proj cols of head0, rows 64:96 = head1, rest 0
        wp_np = np.zeros((C, C), np.float32)
        wp_np[0:32, :] = w_proj[:, 64 * pi : 64 * pi + 32].T
        wp_np[64:96, :] = w_proj[:, 64 * pi + 32 : 64 * pi + 64].T
        # v-bias folds through softmax (rows sum to 1) into the projection bias
        vb_sub = np.concatenate(
            [bf[2 * C + CH * g : 2 * C + CH * (g + 1)] for g in hg]
        )
        hb_np = (
            0.5 * b_proj + w_proj[:, 64 * pi : 64 * (pi + 1)] @ vb_sub
        ).reshape(C, 1).astype(np.float32)

        in_maps.append(
            {
                "x": x[b].reshape(C, L),
                "wqk": wqk_np.astype(ml_dtypes.bfloat16),
                "wv": wv_np.astype(ml_dtypes.bfloat16),
                "bqk": bqk_np,
                "bv": bv_np,
                "wp": wp_np.astype(ml_dtypes.bfloat16),
                "hb": hb_np,
                "gmat": gmat_np,
            }
        )
    return in_maps


def combine_outputs(results):
    out = np.empty((B, C, H, W), np.float32)
    for b in range(B):
        s = results[2 * b]["part"] + results[2 * b + 1]["part"]
        out[b] = s.reshape(C, H, W)
    return out


def _ensure_ntff_hook():
    """Register the axon NTFF profile hook if the environment lacks antenv.axon_hooks."""
    import types, contextlib, ctypes, os

    try:
        import antenv.axon_hooks  # noqa: F401
        return
    except ImportError:
        pass
    mod = types.ModuleType("antenv.axon_hooks")
    state = {"hook": None}
    mod.set_axon_ntff_profile_hook = lambda h: state.__setitem__("hook", h)
    mod.get_axon_ntff_profile_hook = lambda: state["hook"]
    sys.modules["antenv.axon_hooks"] = mod

    so_path = "/opt/axon/libaxon_pjrt.so"
    if not os.path.exists(so_path):
        return
    lib = ctypes.CDLL(so_path)
    if not hasattr(lib, "axon_start_nrt_profile"):
        return
    lib.axon_start_nrt_profile.argtypes = [ctypes.POINTER(ctypes.c_int64), ctypes.c_size_t]
    lib.axon_start_nrt_profile.restype = ctypes.c_int64
    lib.axon_stop_nrt_profile.argtypes = [ctypes.c_char_p]
    lib.axon_stop_nrt_profile.restype = ctypes.c_int64

    @contextlib.contextmanager
    def _hook(output_dir, device_ids):
        import jax

        jax.devices()
        if device_ids:
            ids = (ctypes.c_int64 * len(device_ids))(*device_ids)
            rc = lib.axon_start_nrt_profile(ids, len(device_ids))
        else:
            rc = lib.axon_start_nrt_profile(None, 0)
        if rc != 0:
            raise RuntimeError(f"axon_start_nrt_profile rc={rc}")
        try:
            yield
        finally:
            n = lib.axon_stop_nrt_profile(str(output_dir).encode())
            print(f"profile: {n} file(s) written to {output_dir}", file=sys.stderr)

    state["hook"] = _hook


def kernel_run(inputs, trace=False):
    nc = _build_program()
    in_maps = make_in_maps(inputs)
    if trace:
        _ensure_ntff_hook()
    res = run_bass_kernel_spmd(nc, in_maps, core_ids=list(range(NCORES)), trace=trace)
    return combine_outputs(res.results), res


def kernel(**inputs) -> np.ndarray:
    out, _ = kernel_run(inputs)
    return out

